# revision 10
# baseline (speedup 1.0000x reference)

# Trainium2 Bass kernel for MinConvExpLSTMCell (v7).
#
# Math (linear-space reformulation of the reference's log-space scan):
#   y = conv3x3(x, W) + b; [f_gate, i_gate, h_tilde] = split(y)
#   diff = f_gate - i_gate = conv(x, W_f - W_i) + (b_f - b_i)
#   f = sigmoid(diff);  i = 1 - f = sigmoid(-diff)
#   g = min(sigmoid(y), 0.5) + relu(y),  y = h_tilde + b_h
#     (sigmoid(min(y,0)) == min(sigmoid(y),0.5) by monotonicity - exact)
#   h_t = f_t * h_{t-1} + i_t * g_t,  h_{-1} = g(h0)
#
# Sharding: 8 cores = 4 batches x 2 spatial halves (16 output rows each).
#
# Matmul: K=128 tap-pair packing - x stored twice in SBUF (partitions
# 0:63 "copy A", partitions 64:127 shifted down one image row "copy B"),
# one K=128 matmul contracts two vertically-adjacent taps at once.
# 2x column tiling gives pixel-split psum (partitions 0:63 = rows 0:8 of
# the half-image, 64:127 = rows 8:16) so post runs on 128 partitions.
#
# Software-pipelined emission over 16 four-step groups: each group's
# psum eviction (ACT: f/i sigmoids; DVE: m/r dual-op) is queued before
# older segments' tail work, and the per-segment scan is staggered one
# pixel-chunk per group so the FIFO engine queues never convoy the
# tensor engine's psum turnover. Scan operands are bf16 (fp32 state).

import sys
import numpy as np

sys.path.insert(0, "/opt/trn_rl_repo")

import ml_dtypes
from contextlib import ExitStack

import concourse.bass as bass
import concourse.bacc as bacc
import concourse.mybir as mybir
from concourse.tile import TileContext
from concourse.bass_utils import run_bass_kernel_spmd

BF16 = ml_dtypes.bfloat16
B, T, C, H, W = 4, 64, 64, 32, 32
SEG = 16
NSEG = T // SEG            # 4
NHF = SEG // 4             # 4-step psum groups per segment
HP, WP = 18, 34            # padded shard rows/cols
RC = HP * WP               # 612
RCE = RC + WP              # 646: one extra zero row for shifted copy B
PXH = 256                  # pixels per column-strip (8 rows x 32 cols)
TS = SEG + 1               # scan slots per pixel per segment
NF = PXH * TS              # 4352 scan free size
DNS = SEG * PXH            # 4096 dense free size
NCH = 4                    # scan pixel-chunks per segment
CPX = PXH // NCH           # 64 pixels per chunk

_CACHE = {}


def _build():
    f32 = mybir.dt.float32
    bf16 = mybir.dt.bfloat16
    AF = mybir.ActivationFunctionType
    OP = mybir.AluOpType

    nc = bacc.Bacc()
    xs = nc.dram_tensor("xs", [T, C, RCE], bf16, kind="ExternalInput")
    wt = nc.dram_tensor("wt", [128, 768], bf16, kind="ExternalInput")
    cst = nc.dram_tensor("cst", [128, 3 + PXH], f32, kind="ExternalInput")
    out = nc.dram_tensor("out", [NSEG, 128, NF], bf16, kind="ExternalOutput")

    with TileContext(nc) as tc, ExitStack() as ctx:
        cpool = ctx.enter_context(tc.tile_pool(name="consts", bufs=1))
        xpool = ctx.enter_context(tc.tile_pool(name="x", bufs=2))
        pspool = ctx.enter_context(tc.tile_pool(name="ps", bufs=2, space="PSUM"))
        sigpool = ctx.enter_context(tc.tile_pool(name="sig", bufs=2))
        rpool = ctx.enter_context(tc.tile_pool(name="r", bufs=2))
        spool = ctx.enter_context(tc.tile_pool(name="s", bufs=2))
        ipool = ctx.enter_context(tc.tile_pool(name="i", bufs=2))
        upool = ctx.enter_context(tc.tile_pool(name="u", bufs=2))
        hpool = ctx.enter_context(tc.tile_pool(name="h", bufs=3))

        w_sb = cpool.tile([128, 768], bf16)
        nc.sync.dma_start(w_sb[:, :], wt[:, :])
        cst_sb = cpool.tile([128, 3 + PXH], f32)
        nc.sync.dma_start(cst_sb[:, :], cst[:, :])
        bd2 = cst_sb[:, 0:1]
        bh2 = cst_sb[:, 1:2]
        nbd2 = cst_sb[:, 2:3]
        g0c = cst_sb[:, 3:3 + PXH]

        seg = {}          # per-segment tile context

        def alloc_segment(s):
            xt = xpool.tile([128, SEG * RC], bf16, name="xt")
            xtv = xt.rearrange("p (t x) -> p t x", t=SEG)
            for q in range(NHF):
                tq = slice(q * 4, (q + 1) * 4)
                srcA = xs[s * SEG + q * 4:s * SEG + (q + 1) * 4, :, 0:RC]
                nc.sync.dma_start(xtv[0:64, tq], srcA.rearrange("t c x -> c t x"))
                srcB = xs[s * SEG + q * 4:s * SEG + (q + 1) * 4, :, WP:WP + RC]
                nc.sync.dma_start(xtv[64:128, tq], srcB.rearrange("t c x -> c t x"))
            SIGt = sigpool.tile([128, NF], bf16, name="SIGt")
            Ut = upool.tile([128, NF], bf16, name="Ut")
            Ht = hpool.tile([128, NF], bf16, name="Ht")
            Rt = rpool.tile([128, DNS], bf16, name="Rt")
            SYt = spool.tile([128, DNS], bf16, name="SYt")
            It = ipool.tile([128, DNS], bf16, name="It")
            sigv = SIGt.rearrange("p (x t) -> p x t", t=TS)
            uv = Ut.rearrange("p (x t) -> p x t", t=TS)
            nc.gpsimd.memset(sigv[:, :, 0], 0.0)
            seg[s] = dict(xt=xt, SIGt=SIGt, Ut=Ut, Ht=Ht, Rt=Rt,
                          SYt=SYt, It=It, sigv=sigv, uv=uv)

        def emit_col0(s):
            c = seg[s]
            if s == 0:
                nc.vector.tensor_copy(c["uv"][:, :, 0], g0c)
            else:
                hv = seg[s - 1]["Ht"].rearrange("p (x t) -> p x t", t=TS)
                nc.vector.tensor_copy(c["uv"][:, :, 0], hv[:, :, SEG])

        def emit_scan_chunk(s, k):
            c = seg[s]
            a, b2 = k * CPX * TS, (k + 1) * CPX * TS
            nc.vector.tensor_tensor_scan(
                c["Ht"][:, a:b2], c["SIGt"][:, a:b2], c["Ut"][:, a:b2],
                0.0, OP.mult, OP.add)
            nc.sync.dma_start(out[s, :, a:b2], c["Ht"][:, a:b2])

        ngroups = NSEG * NHF
        for g in range(ngroups):
            s, hf = divmod(g, NHF)
            if hf == 0:
                alloc_segment(s)
                if s == 0:
                    emit_col0(0)
            c = seg[s]
            xv = c["xt"].rearrange("p (t r c) -> p t r c", t=SEG, r=HP, c=WP)

            ps = pspool.tile([128, 2048], f32, name="ps")
            for kappa in range(2):           # 0 = diff, 1 = h_tilde
                po = kappa * 1024
                for p in range(6):
                    blk = (kappa * 6 + p) * 64
                    lhsT = w_sb[:, blk:blk + 64]
                    r0, c0 = (0, p) if p < 3 else (2, p - 3)
                    for sp in range(2):
                        t0 = hf * 4 + sp * 2
                        for cs_ in range(2):
                            rhs = xv[0:128, t0:t0 + 2,
                                     r0 + 8 * cs_:r0 + 8 * cs_ + 8,
                                     c0:c0 + 32]
                            dst = ps[cs_ * 64:(cs_ + 1) * 64,
                                     po + sp * 512:po + sp * 512 + 512]
                            nc.tensor.matmul(
                                dst, lhsT, rhs,
                                start=(p == 0), stop=(p == 5),
                                tile_position=(0, cs_ * 64))

            sl = slice(hf * 1024, (hf + 1) * 1024)
            pdq = ps[:, 0:1024].rearrange("p (a s x) -> p a x s",
                                          a=2, s=2, x=PXH)
            phq = ps[:, 1024:2048].rearrange("p (a s x) -> p a x s",
                                             a=2, s=2, x=PXH)
            # psum eviction: f, i on ACT; m, r on DVE (bias folded)
            sdst = c["sigv"][:, :, 1 + hf * 4:5 + hf * 4].rearrange(
                "p x (a s) -> p a x s", a=2, s=2)
            nc.scalar.activation(sdst, pdq, AF.Sigmoid, bias=bd2)
            nc.scalar.activation(
                c["It"][:, sl], pdq, AF.Sigmoid, bias=nbd2, scale=-1.0)
            # sigy = sigmoid(ht + bh) on ACT; r = relu(ht + bh) on DVE
            nc.scalar.activation(
                c["SYt"][:, sl], phq, AF.Sigmoid, bias=bh2)
            rq = c["Rt"][:, sl].rearrange("p (a x s) -> p a x s",
                                          a=2, x=PXH, s=2)
            nc.vector.tensor_scalar(rq, phq, bh2, 0.0, OP.add, OP.max)
            # g = min(sigy, 0.5) + r  (one fused op, in place on Rt)
            nc.vector.scalar_tensor_tensor(
                c["Rt"][:, sl], c["SYt"][:, sl], 0.5, c["Rt"][:, sl],
                OP.min, OP.add)
            udst = c["uv"][:, :, 1 + hf * 4:5 + hf * 4].rearrange(
                "p x (a s) -> p a x s", a=2, s=2)
            ivq = c["It"][:, sl].rearrange("p (a x s) -> p a x s",
                                           a=2, x=PXH, s=2)
            gvq = c["Rt"][:, sl].rearrange("p (a x s) -> p a x s",
                                           a=2, x=PXH, s=2)
            nc.gpsimd.tensor_tensor(udst, ivq, gvq, OP.mult)

            # staggered: one scan chunk of the previous segment per group
            if s >= 1:
                emit_scan_chunk(s - 1, hf)
                if hf == NHF - 1:
                    emit_col0(s)      # needs all of Ht(s-1)

        for k in range(NCH):
            emit_scan_chunk(NSEG - 1, k)
    nc.finalize()
    return nc


def _g0(h0):
    return np.where(h0 >= 0.0, h0 + 0.5, 1.0 / (1.0 + np.exp(-h0))).astype(np.float32)


def kernel(x, conv_w, conv_b, h0):
    x = np.asarray(x, np.float32)
    conv_w = np.asarray(conv_w, np.float32)
    conv_b = np.asarray(conv_b, np.float32)
    h0 = np.asarray(h0, np.float32)

    if "nc" not in _CACHE:
        _CACHE["nc"] = _build()
    nc = _CACHE["nc"]

    wd = conv_w[0:64] - conv_w[64:128]           # [64out, 64in, 3, 3]
    wh = conv_w[128:192]
    bd = conv_b[0:64] - conv_b[64:128]
    bh = conv_b[128:192]

    wt = np.zeros((128, 768), np.float32)
    for kappa, wk in ((0, wd), (1, wh)):
        for p in range(6):
            blk = (kappa * 6 + p) * 64
            if p < 3:
                # paired taps (0,p) on rows 0:64, (1,p) on rows 64:128
                wt[0:64, blk:blk + 64] = wk[:, :, 0, p].T
                wt[64:128, blk:blk + 64] = wk[:, :, 1, p].T
            else:
                # single tap (2, p-3); rows 64:128 stay zero
                wt[0:64, blk:blk + 64] = wk[:, :, 2, p - 3].T
    wt = wt.astype(BF16)

    x4 = x.reshape(B, T, C, H, W)
    g0f = _g0(h0)                                 # [B, C, H, W]

    bd2 = np.concatenate([bd, bd])[:, None]
    bh2 = np.concatenate([bh, bh])[:, None]

    in_maps = []
    for c in range(8):
        b, half = c // 2, c % 2
        xsh = np.zeros((T, C, HP + 1, WP), np.float32)
        if half == 0:
            xsh[:, :, 1:18, 1:33] = x4[b, :, :, 0:17, :]
        else:
            xsh[:, :, 0:17, 1:33] = x4[b, :, :, 15:32, :]
        xsh = xsh.reshape(T, C, RCE).astype(BF16)
        r16 = g0f[b, :, 16 * half:16 * half + 16, :]     # [64, 16, 32]
        g0c = np.concatenate(
            [r16[:, 0:8, :].reshape(64, PXH),
             r16[:, 8:16, :].reshape(64, PXH)], 0)       # [128, 256]
        cstc = np.concatenate(
            [bd2, bh2, -bd2, g0c], 1).astype(np.float32)
        in_maps.append({"xs": xsh, "wt": wt, "cst": cstc})

    _CACHE["in_maps"] = in_maps
    res = run_bass_kernel_spmd(nc, in_maps, core_ids=list(range(8)))

    outf = np.empty((B, T, C, H, W), np.float32)
    for c in range(8):
        b, half = c // 2, c % 2
        arr = np.asarray(res.results[c]["out"], np.float32)
        arr = arr.reshape(NSEG, 128, PXH, TS)[:, :, :, 1:]
        arr = arr.transpose(0, 3, 1, 2).reshape(T, 128, 8, 32)
        outf[b, :, :, 16 * half:16 * half + 8, :] = arr[:, 0:64]
        outf[b, :, :, 16 * half + 8:16 * half + 16, :] = arr[:, 64:128]
    return outf.reshape(B * T, C, H, W)


# revision 11
# speedup vs baseline: 1.1579x; 1.1579x over previous

# Trainium2 Bass kernel for MinConvExpLSTMCell (v7).
#
# Math (linear-space reformulation of the reference's log-space scan):
#   y = conv3x3(x, W) + b; [f_gate, i_gate, h_tilde] = split(y)
#   diff = f_gate - i_gate = conv(x, W_f - W_i) + (b_f - b_i)
#   f = sigmoid(diff);  i = 1 - f = sigmoid(-diff)
#   g = min(sigmoid(y), 0.5) + relu(y),  y = h_tilde + b_h
#     (sigmoid(min(y,0)) == min(sigmoid(y),0.5) by monotonicity - exact)
#   h_t = f_t * h_{t-1} + i_t * g_t,  h_{-1} = g(h0)
#
# Sharding: 8 cores = 4 batches x 2 spatial halves (16 output rows each).
#
# Matmul: K=128 tap-pair packing - x stored twice in SBUF (partitions
# 0:63 "copy A", partitions 64:127 shifted down one image row "copy B"),
# one K=128 matmul contracts two vertically-adjacent taps at once.
# 2x column tiling gives pixel-split psum (partitions 0:63 = rows 0:8 of
# the half-image, 64:127 = rows 8:16) so post runs on 128 partitions.
#
# Software-pipelined emission over 16 four-step groups: each group's
# psum eviction (ACT: f/i sigmoids; DVE: m/r dual-op) is queued before
# older segments' tail work, and the per-segment scan is staggered one
# pixel-chunk per group so the FIFO engine queues never convoy the
# tensor engine's psum turnover. Scan operands are bf16 (fp32 state).

import sys
import numpy as np

sys.path.insert(0, "/opt/trn_rl_repo")

import ml_dtypes
from contextlib import ExitStack

import concourse.bass as bass
import concourse.bacc as bacc
import concourse.mybir as mybir
from concourse.tile import TileContext
from concourse.bass_utils import run_bass_kernel_spmd

BF16 = ml_dtypes.bfloat16
B, T, C, H, W = 4, 64, 64, 32, 32
SEG = 16
NSEG = T // SEG            # 4
NHF = SEG // 4             # 4-step psum groups per segment
HP, WP = 18, 34            # padded shard rows/cols
RC = HP * WP               # 612
RCE = RC + WP              # 646: one extra zero row for shifted copy B
PXH = 256                  # pixels per column-strip (8 rows x 32 cols)
TS = SEG + 1               # scan slots per pixel per segment
NF = PXH * TS              # 4352 scan free size
DNS = SEG * PXH            # 4096 dense free size
NCH = 4                    # scan pixel-chunks per segment
CPX = PXH // NCH           # 64 pixels per chunk

_CACHE = {}


def _build():
    f32 = mybir.dt.float32
    bf16 = mybir.dt.bfloat16
    AF = mybir.ActivationFunctionType
    OP = mybir.AluOpType

    nc = bacc.Bacc()
    xs = nc.dram_tensor("xs", [T, C, RCE], bf16, kind="ExternalInput")
    wt = nc.dram_tensor("wt", [128, 768], bf16, kind="ExternalInput")
    cst = nc.dram_tensor("cst", [128, 3 + PXH], f32, kind="ExternalInput")
    out = nc.dram_tensor("out", [NSEG, 128, NF], bf16, kind="ExternalOutput")

    with TileContext(nc) as tc, ExitStack() as ctx:
        cpool = ctx.enter_context(tc.tile_pool(name="consts", bufs=1))
        xpool = ctx.enter_context(tc.tile_pool(name="x", bufs=2))
        pspool = ctx.enter_context(tc.tile_pool(name="ps", bufs=2, space="PSUM"))
        sigpool = ctx.enter_context(tc.tile_pool(name="sig", bufs=3))
        rpool = ctx.enter_context(tc.tile_pool(name="r", bufs=2))
        spool = ctx.enter_context(tc.tile_pool(name="s", bufs=2))
        ipool = ctx.enter_context(tc.tile_pool(name="i", bufs=2))
        upool = ctx.enter_context(tc.tile_pool(name="u", bufs=3))
        hpool = ctx.enter_context(tc.tile_pool(name="h", bufs=3))

        w_sb = cpool.tile([128, 768], bf16)
        nc.sync.dma_start(w_sb[:, :], wt[:, :])
        cst_sb = cpool.tile([128, 3 + PXH], f32)
        nc.sync.dma_start(cst_sb[:, :], cst[:, :])
        bd2 = cst_sb[:, 0:1]
        bh2 = cst_sb[:, 1:2]
        nbd2 = cst_sb[:, 2:3]
        g0c = cst_sb[:, 3:3 + PXH]

        seg = {}          # per-segment tile context

        def alloc_segment(s):
            xt = xpool.tile([128, SEG * RC], bf16, name="xt")
            xtv = xt.rearrange("p (t x) -> p t x", t=SEG)
            for q in range(NHF):
                tq = slice(q * 4, (q + 1) * 4)
                srcA = xs[s * SEG + q * 4:s * SEG + (q + 1) * 4, :, 0:RC]
                nc.sync.dma_start(xtv[0:64, tq], srcA.rearrange("t c x -> c t x"))
                srcB = xs[s * SEG + q * 4:s * SEG + (q + 1) * 4, :, WP:WP + RC]
                nc.sync.dma_start(xtv[64:128, tq], srcB.rearrange("t c x -> c t x"))
            SIGt = sigpool.tile([128, NF], bf16, name="SIGt")
            Ut = upool.tile([128, NF], bf16, name="Ut")
            Ht = hpool.tile([128, NF], bf16, name="Ht")
            Rt = rpool.tile([128, DNS], bf16, name="Rt")
            SYt = spool.tile([128, DNS], bf16, name="SYt")
            St = spool.tile([128, DNS], bf16, name="St")
            It = ipool.tile([128, DNS], bf16, name="It")
            sigv = SIGt.rearrange("p (x t) -> p x t", t=TS)
            uv = Ut.rearrange("p (x t) -> p x t", t=TS)
            nc.gpsimd.memset(sigv[:, :, 0], 0.0)
            seg[s] = dict(xt=xt, SIGt=SIGt, Ut=Ut, Ht=Ht, Rt=Rt,
                          SYt=SYt, St=St, It=It, sigv=sigv, uv=uv)

        def emit_col0(s):
            c = seg[s]
            if s == 0:
                nc.vector.tensor_copy(c["uv"][:, :, 0], g0c)
            else:
                hv = seg[s - 1]["Ht"].rearrange("p (x t) -> p x t", t=TS)
                nc.vector.tensor_copy(c["uv"][:, :, 0], hv[:, :, SEG])

        def emit_scan_chunk(s, k):
            c = seg[s]
            a, b2 = k * CPX * TS, (k + 1) * CPX * TS
            nc.vector.tensor_tensor_scan(
                c["Ht"][:, a:b2], c["SIGt"][:, a:b2], c["Ut"][:, a:b2],
                0.0, OP.mult, OP.add)
            nc.sync.dma_start(out[s, :, a:b2], c["Ht"][:, a:b2])

        ngroups = NSEG * NHF
        alloc_segment(0)
        emit_col0(0)
        for g in range(ngroups):
            s, hf = divmod(g, NHF)
            if hf == NHF - 2 and s + 1 < NSEG:
                alloc_segment(s + 1)
            c = seg[s]
            xv = c["xt"].rearrange("p (t r c) -> p t r c", t=SEG, r=HP, c=WP)

            ps = pspool.tile([128, 2048], f32, name="ps")
            for kappa in range(2):           # 0 = diff, 1 = h_tilde
                po = kappa * 1024
                for p in range(6):
                    blk = (kappa * 6 + p) * 64
                    lhsT = w_sb[:, blk:blk + 64]
                    r0, c0 = (0, p) if p < 3 else (2, p - 3)
                    for sp in range(2):
                        t0 = hf * 4 + sp * 2
                        for cs_ in range(2):
                            rhs = xv[0:128, t0:t0 + 2,
                                     r0 + 8 * cs_:r0 + 8 * cs_ + 8,
                                     c0:c0 + 32]
                            dst = ps[cs_ * 64:(cs_ + 1) * 64,
                                     po + sp * 512:po + sp * 512 + 512]
                            nc.tensor.matmul(
                                dst, lhsT, rhs,
                                start=(p == 0), stop=(p == 5),
                                tile_position=(0, cs_ * 64))

            sl = slice(hf * 1024, (hf + 1) * 1024)
            pdq = ps[:, 0:1024].rearrange("p (a s x) -> p a x s",
                                          a=2, s=2, x=PXH)
            phq = ps[:, 1024:2048].rearrange("p (a s x) -> p a x s",
                                             a=2, s=2, x=PXH)
            # psum eviction: f, i on ACT; m, r on DVE (bias folded)
            sdst = c["sigv"][:, :, 1 + hf * 4:5 + hf * 4].rearrange(
                "p x (a s) -> p a x s", a=2, s=2)
            nc.scalar.activation(sdst, pdq, AF.Sigmoid, bias=bd2)
            nc.scalar.activation(
                c["It"][:, sl], pdq, AF.Sigmoid, bias=nbd2, scale=-1.0)
            # sigy = sigmoid(ht + bh) on ACT; r = relu(ht + bh) on DVE
            nc.scalar.activation(
                c["SYt"][:, sl], phq, AF.Sigmoid, bias=bh2)
            rq = c["Rt"][:, sl].rearrange("p (a x s) -> p a x s",
                                          a=2, x=PXH, s=2)
            nc.vector.tensor_scalar(rq, phq, bh2, 0.0, OP.add, OP.max)
            # s = min(sigy, 0.5) on DVE (4x); g = s + r on gpsimd
            nc.vector.tensor_scalar(
                c["St"][:, sl], c["SYt"][:, sl], 0.5, None, OP.min)
            nc.gpsimd.tensor_tensor(
                c["Rt"][:, sl], c["St"][:, sl], c["Rt"][:, sl], OP.add)
            udst = c["uv"][:, :, 1 + hf * 4:5 + hf * 4].rearrange(
                "p x (a s) -> p a x s", a=2, s=2)
            ivq = c["It"][:, sl].rearrange("p (a x s) -> p a x s",
                                           a=2, x=PXH, s=2)
            gvq = c["Rt"][:, sl].rearrange("p (a x s) -> p a x s",
                                           a=2, x=PXH, s=2)
            nc.gpsimd.tensor_tensor(udst, ivq, gvq, OP.mult)

            # staggered: one scan chunk of the previous segment per group
            if s >= 1:
                emit_scan_chunk(s - 1, hf)
                if hf == NHF - 1:
                    emit_col0(s)      # needs all of Ht(s-1)

        for k in range(NCH):
            emit_scan_chunk(NSEG - 1, k)
    nc.finalize()
    return nc


def _g0(h0):
    return np.where(h0 >= 0.0, h0 + 0.5, 1.0 / (1.0 + np.exp(-h0))).astype(np.float32)


def kernel(x, conv_w, conv_b, h0):
    x = np.asarray(x, np.float32)
    conv_w = np.asarray(conv_w, np.float32)
    conv_b = np.asarray(conv_b, np.float32)
    h0 = np.asarray(h0, np.float32)

    if "nc" not in _CACHE:
        _CACHE["nc"] = _build()
    nc = _CACHE["nc"]

    wd = conv_w[0:64] - conv_w[64:128]           # [64out, 64in, 3, 3]
    wh = conv_w[128:192]
    bd = conv_b[0:64] - conv_b[64:128]
    bh = conv_b[128:192]

    wt = np.zeros((128, 768), np.float32)
    for kappa, wk in ((0, wd), (1, wh)):
        for p in range(6):
            blk = (kappa * 6 + p) * 64
            if p < 3:
                # paired taps (0,p) on rows 0:64, (1,p) on rows 64:128
                wt[0:64, blk:blk + 64] = wk[:, :, 0, p].T
                wt[64:128, blk:blk + 64] = wk[:, :, 1, p].T
            else:
                # single tap (2, p-3); rows 64:128 stay zero
                wt[0:64, blk:blk + 64] = wk[:, :, 2, p - 3].T
    wt = wt.astype(BF16)

    x4 = x.reshape(B, T, C, H, W)
    g0f = _g0(h0)                                 # [B, C, H, W]

    bd2 = np.concatenate([bd, bd])[:, None]
    bh2 = np.concatenate([bh, bh])[:, None]

    in_maps = []
    for c in range(8):
        b, half = c // 2, c % 2
        xsh = np.zeros((T, C, HP + 1, WP), np.float32)
        if half == 0:
            xsh[:, :, 1:18, 1:33] = x4[b, :, :, 0:17, :]
        else:
            xsh[:, :, 0:17, 1:33] = x4[b, :, :, 15:32, :]
        xsh = xsh.reshape(T, C, RCE).astype(BF16)
        r16 = g0f[b, :, 16 * half:16 * half + 16, :]     # [64, 16, 32]
        g0c = np.concatenate(
            [r16[:, 0:8, :].reshape(64, PXH),
             r16[:, 8:16, :].reshape(64, PXH)], 0)       # [128, 256]
        cstc = np.concatenate(
            [bd2, bh2, -bd2, g0c], 1).astype(np.float32)
        in_maps.append({"xs": xsh, "wt": wt, "cst": cstc})

    _CACHE["in_maps"] = in_maps
    res = run_bass_kernel_spmd(nc, in_maps, core_ids=list(range(8)))

    outf = np.empty((B, T, C, H, W), np.float32)
    for c in range(8):
        b, half = c // 2, c % 2
        arr = np.asarray(res.results[c]["out"], np.float32)
        arr = arr.reshape(NSEG, 128, PXH, TS)[:, :, :, 1:]
        arr = arr.transpose(0, 3, 1, 2).reshape(T, 128, 8, 32)
        outf[b, :, :, 16 * half:16 * half + 8, :] = arr[:, 0:64]
        outf[b, :, :, 16 * half + 8:16 * half + 16, :] = arr[:, 64:128]
    return outf.reshape(B * T, C, H, W)


# revision 12
# speedup vs baseline: 1.1859x; 1.0242x over previous

# Trainium2 Bass kernel for MinConvExpLSTMCell (v7).
#
# Math (linear-space reformulation of the reference's log-space scan):
#   y = conv3x3(x, W) + b; [f_gate, i_gate, h_tilde] = split(y)
#   diff = f_gate - i_gate = conv(x, W_f - W_i) + (b_f - b_i)
#   f = sigmoid(diff);  i = 1 - f = sigmoid(-diff)
#   g = min(sigmoid(y), 0.5) + relu(y),  y = h_tilde + b_h
#     (sigmoid(min(y,0)) == min(sigmoid(y),0.5) by monotonicity - exact)
#   h_t = f_t * h_{t-1} + i_t * g_t,  h_{-1} = g(h0)
#
# Sharding: 8 cores = 4 batches x 2 spatial halves (16 output rows each).
#
# Matmul: K=128 tap-pair packing - x stored twice in SBUF (partitions
# 0:63 "copy A", partitions 64:127 shifted down one image row "copy B"),
# one K=128 matmul contracts two vertically-adjacent taps at once.
# 2x column tiling gives pixel-split psum (partitions 0:63 = rows 0:8 of
# the half-image, 64:127 = rows 8:16) so post runs on 128 partitions.
#
# Software-pipelined emission over 16 four-step groups: each group's
# psum eviction (ACT: f/i sigmoids; DVE: m/r dual-op) is queued before
# older segments' tail work, and the per-segment scan is staggered one
# pixel-chunk per group so the FIFO engine queues never convoy the
# tensor engine's psum turnover. Scan operands are bf16 (fp32 state).

import sys
import numpy as np

sys.path.insert(0, "/opt/trn_rl_repo")

import ml_dtypes
from contextlib import ExitStack

import concourse.bass as bass
import concourse.bacc as bacc
import concourse.mybir as mybir
from concourse.tile import TileContext
from concourse.bass_utils import run_bass_kernel_spmd

BF16 = ml_dtypes.bfloat16
B, T, C, H, W = 4, 64, 64, 32, 32
SEG = 16
NSEG = T // SEG            # 4
NHF = SEG // 4             # 4-step psum groups per segment
HP, WP = 18, 34            # padded shard rows/cols
RC = HP * WP               # 612
RCE = RC + WP              # 646: one extra zero row for shifted copy B
PXH = 256                  # pixels per column-strip (8 rows x 32 cols)
TS = SEG + 1               # scan slots per pixel per segment
NF = PXH * TS              # 4352 scan free size
DNS = SEG * PXH            # 4096 dense free size
NCH = 4                    # scan pixel-chunks per segment
CPX = PXH // NCH           # 64 pixels per chunk

_CACHE = {}


def _build():
    f32 = mybir.dt.float32
    bf16 = mybir.dt.bfloat16
    AF = mybir.ActivationFunctionType
    OP = mybir.AluOpType

    nc = bacc.Bacc()
    xs = nc.dram_tensor("xs", [T, C, RCE], bf16, kind="ExternalInput")
    wt = nc.dram_tensor("wt", [128, 768], bf16, kind="ExternalInput")
    cst = nc.dram_tensor("cst", [128, 3 + PXH], f32, kind="ExternalInput")
    out = nc.dram_tensor("out", [NSEG, 128, NF], bf16, kind="ExternalOutput")

    with TileContext(nc) as tc, ExitStack() as ctx:
        cpool = ctx.enter_context(tc.tile_pool(name="consts", bufs=1))
        xpool = ctx.enter_context(tc.tile_pool(name="x", bufs=2))
        pspool = ctx.enter_context(tc.tile_pool(name="ps", bufs=2, space="PSUM"))
        sigpool = ctx.enter_context(tc.tile_pool(name="sig", bufs=4))
        rpool = ctx.enter_context(tc.tile_pool(name="r", bufs=2))
        spool = ctx.enter_context(tc.tile_pool(name="s", bufs=2))
        ipool = ctx.enter_context(tc.tile_pool(name="i", bufs=2))
        upool = ctx.enter_context(tc.tile_pool(name="u", bufs=4))
        hpool = ctx.enter_context(tc.tile_pool(name="h", bufs=4))

        w_sb = cpool.tile([128, 768], bf16)
        nc.sync.dma_start(w_sb[:, :], wt[:, :])
        cst_sb = cpool.tile([128, 3 + PXH], f32)
        nc.sync.dma_start(cst_sb[:, :], cst[:, :])
        bd2 = cst_sb[:, 0:1]
        bh2 = cst_sb[:, 1:2]
        nbd2 = cst_sb[:, 2:3]
        g0c = cst_sb[:, 3:3 + PXH]

        seg = {}          # per-segment tile context

        def alloc_segment(s):
            xt = xpool.tile([128, SEG * RC], bf16, name="xt")
            xtv = xt.rearrange("p (t x) -> p t x", t=SEG)
            for q in range(NHF):
                tq = slice(q * 4, (q + 1) * 4)
                srcA = xs[s * SEG + q * 4:s * SEG + (q + 1) * 4, :, 0:RC]
                nc.sync.dma_start(xtv[0:64, tq], srcA.rearrange("t c x -> c t x"))
                srcB = xs[s * SEG + q * 4:s * SEG + (q + 1) * 4, :, WP:WP + RC]
                nc.sync.dma_start(xtv[64:128, tq], srcB.rearrange("t c x -> c t x"))
            SIGt = sigpool.tile([128, NF], bf16, name="SIGt")
            Ut = upool.tile([128, NF], bf16, name="Ut")
            Ht = hpool.tile([128, NF], bf16, name="Ht")
            Rt = rpool.tile([128, DNS], bf16, name="Rt")
            SYt = spool.tile([128, DNS], bf16, name="SYt")
            It = ipool.tile([128, DNS], bf16, name="It")
            sigv = SIGt.rearrange("p (x t) -> p x t", t=TS)
            uv = Ut.rearrange("p (x t) -> p x t", t=TS)
            nc.gpsimd.memset(sigv[:, :, 0], 0.0)
            seg[s] = dict(xt=xt, SIGt=SIGt, Ut=Ut, Ht=Ht, Rt=Rt,
                          SYt=SYt, It=It, sigv=sigv, uv=uv)

        def emit_col0(s):
            c = seg[s]
            if s == 0:
                nc.vector.tensor_copy(c["uv"][:, :, 0], g0c)
            else:
                hv = seg[s - 1]["Ht"].rearrange("p (x t) -> p x t", t=TS)
                nc.vector.tensor_copy(c["uv"][:, :, 0], hv[:, :, SEG])

        def emit_scan_chunk(s, k):
            c = seg[s]
            a, b2 = k * CPX * TS, (k + 1) * CPX * TS
            nc.vector.tensor_tensor_scan(
                c["Ht"][:, a:b2], c["SIGt"][:, a:b2], c["Ut"][:, a:b2],
                0.0, OP.mult, OP.add)
            nc.sync.dma_start(out[s, :, a:b2], c["Ht"][:, a:b2])

        ngroups = NSEG * NHF
        alloc_segment(0)
        emit_col0(0)
        for g in range(ngroups):
            s, hf = divmod(g, NHF)
            if hf == 0 and s + 1 < NSEG:
                alloc_segment(s + 1)
            c = seg[s]
            xv = c["xt"].rearrange("p (t r c) -> p t r c", t=SEG, r=HP, c=WP)

            ps = pspool.tile([128, 2048], f32, name="ps")
            for kappa in range(2):           # 0 = diff, 1 = h_tilde
                po = kappa * 1024
                for p in range(6):
                    blk = (kappa * 6 + p) * 64
                    lhsT = w_sb[:, blk:blk + 64]
                    r0, c0 = (0, p) if p < 3 else (2, p - 3)
                    for sp in range(2):
                        t0 = hf * 4 + sp * 2
                        for cs_ in range(2):
                            rhs = xv[0:128, t0:t0 + 2,
                                     r0 + 8 * cs_:r0 + 8 * cs_ + 8,
                                     c0:c0 + 32]
                            dst = ps[cs_ * 64:(cs_ + 1) * 64,
                                     po + sp * 512:po + sp * 512 + 512]
                            nc.tensor.matmul(
                                dst, lhsT, rhs,
                                start=(p == 0), stop=(p == 5),
                                tile_position=(0, cs_ * 64))

            sl = slice(hf * 1024, (hf + 1) * 1024)
            pdq = ps[:, 0:1024].rearrange("p (a s x) -> p a x s",
                                          a=2, s=2, x=PXH)
            phq = ps[:, 1024:2048].rearrange("p (a s x) -> p a x s",
                                             a=2, s=2, x=PXH)
            # psum eviction: f, i on ACT; m, r on DVE (bias folded)
            sdst = c["sigv"][:, :, 1 + hf * 4:5 + hf * 4].rearrange(
                "p x (a s) -> p a x s", a=2, s=2)
            nc.scalar.activation(sdst, pdq, AF.Sigmoid, bias=bd2)
            nc.scalar.activation(
                c["It"][:, sl], pdq, AF.Sigmoid, bias=nbd2, scale=-1.0)
            # sigy = sigmoid(ht + bh) on ACT; r = relu(ht + bh) on DVE
            nc.scalar.activation(
                c["SYt"][:, sl], phq, AF.Sigmoid, bias=bh2)
            rq = c["Rt"][:, sl].rearrange("p (a x s) -> p a x s",
                                          a=2, x=PXH, s=2)
            nc.vector.tensor_scalar(rq, phq, bh2, 0.0, OP.add, OP.max)
            # s = min(sigy, 0.5) in place on DVE (4x); g = s + r
            nc.vector.tensor_scalar(
                c["SYt"][:, sl], c["SYt"][:, sl], 0.5, None, OP.min)
            geng = nc.vector if g == ngroups - 1 else nc.gpsimd
            geng.tensor_tensor(
                c["Rt"][:, sl], c["SYt"][:, sl], c["Rt"][:, sl], OP.add)
            udst = c["uv"][:, :, 1 + hf * 4:5 + hf * 4].rearrange(
                "p x (a s) -> p a x s", a=2, s=2)
            ivq = c["It"][:, sl].rearrange("p (a x s) -> p a x s",
                                           a=2, x=PXH, s=2)
            gvq = c["Rt"][:, sl].rearrange("p (a x s) -> p a x s",
                                           a=2, x=PXH, s=2)
            geng.tensor_tensor(udst, ivq, gvq, OP.mult)

            # staggered: one scan chunk of the previous segment per group
            if s >= 1:
                emit_scan_chunk(s - 1, hf)
                if hf == NHF - 1:
                    emit_col0(s)      # needs all of Ht(s-1)

        for k in range(NCH):
            emit_scan_chunk(NSEG - 1, k)
    nc.finalize()
    return nc


def _g0(h0):
    return np.where(h0 >= 0.0, h0 + 0.5, 1.0 / (1.0 + np.exp(-h0))).astype(np.float32)


def kernel(x, conv_w, conv_b, h0):
    x = np.asarray(x, np.float32)
    conv_w = np.asarray(conv_w, np.float32)
    conv_b = np.asarray(conv_b, np.float32)
    h0 = np.asarray(h0, np.float32)

    if "nc" not in _CACHE:
        _CACHE["nc"] = _build()
    nc = _CACHE["nc"]

    wd = conv_w[0:64] - conv_w[64:128]           # [64out, 64in, 3, 3]
    wh = conv_w[128:192]
    bd = conv_b[0:64] - conv_b[64:128]
    bh = conv_b[128:192]

    wt = np.zeros((128, 768), np.float32)
    for kappa, wk in ((0, wd), (1, wh)):
        for p in range(6):
            blk = (kappa * 6 + p) * 64
            if p < 3:
                # paired taps (0,p) on rows 0:64, (1,p) on rows 64:128
                wt[0:64, blk:blk + 64] = wk[:, :, 0, p].T
                wt[64:128, blk:blk + 64] = wk[:, :, 1, p].T
            else:
                # single tap (2, p-3); rows 64:128 stay zero
                wt[0:64, blk:blk + 64] = wk[:, :, 2, p - 3].T
    wt = wt.astype(BF16)

    x4 = x.reshape(B, T, C, H, W)
    g0f = _g0(h0)                                 # [B, C, H, W]

    bd2 = np.concatenate([bd, bd])[:, None]
    bh2 = np.concatenate([bh, bh])[:, None]

    in_maps = []
    for c in range(8):
        b, half = c // 2, c % 2
        xsh = np.zeros((T, C, HP + 1, WP), np.float32)
        if half == 0:
            xsh[:, :, 1:18, 1:33] = x4[b, :, :, 0:17, :]
        else:
            xsh[:, :, 0:17, 1:33] = x4[b, :, :, 15:32, :]
        xsh = xsh.reshape(T, C, RCE).astype(BF16)
        r16 = g0f[b, :, 16 * half:16 * half + 16, :]     # [64, 16, 32]
        g0c = np.concatenate(
            [r16[:, 0:8, :].reshape(64, PXH),
             r16[:, 8:16, :].reshape(64, PXH)], 0)       # [128, 256]
        cstc = np.concatenate(
            [bd2, bh2, -bd2, g0c], 1).astype(np.float32)
        in_maps.append({"xs": xsh, "wt": wt, "cst": cstc})

    _CACHE["in_maps"] = in_maps
    res = run_bass_kernel_spmd(nc, in_maps, core_ids=list(range(8)))

    outf = np.empty((B, T, C, H, W), np.float32)
    for c in range(8):
        b, half = c // 2, c % 2
        arr = np.asarray(res.results[c]["out"], np.float32)
        arr = arr.reshape(NSEG, 128, PXH, TS)[:, :, :, 1:]
        arr = arr.transpose(0, 3, 1, 2).reshape(T, 128, 8, 32)
        outf[b, :, :, 16 * half:16 * half + 8, :] = arr[:, 0:64]
        outf[b, :, :, 16 * half + 8:16 * half + 16, :] = arr[:, 64:128]
    return outf.reshape(B * T, C, H, W)
